# revision 6
# baseline (speedup 1.0000x reference)
"""Distributed Trainium2 kernel for the 4-layer single-head causal-attention
stack (returns mean attention weights over layers).

Sharding: sequence-parallel over the 2048 mentions. 16 row-tiles of 128;
core c owns tiles {c, 15-c} so causal-attention work is identical on every
core -> one uniform SPMD program. Per layer each core projects K,V for its
256 rows, all-gathers K,V across the 8 cores (one collective), projects Q
while the gather runs, then computes masked scores, softmax and W@V in
bf16 with f32 PSUM accumulation.

The per-layer output projection is folded into the next layer's QKV
weights on the host (W'_i = Wqkv_i @ Wo_{i-1}), so x_i never
materializes on device - W@V output feeds the next layer's projections
directly, which both removes FLOPs and shortens the serial chain between
the collectives. Layer 3 only needs Q,K (its attention output is never
consumed). Attention weights accumulate in f32; the mean over the 4
layers is the output.
"""

import numpy as np
import ml_dtypes

N, E, L, NCORES = 2048, 1024, 4, 8
EC = E // 128          # 8 contraction chunks of 128
MT = 256               # mention rows per core
SCALE = 1.0 / np.sqrt(np.float32(E))
KV_K_ELEMS = E * MT            # k block: [1024, 256] (feature-major)
KV_V_ELEMS = MT * E            # v block: [256, 1024] (row-major natural)
KV_ELEMS = KV_K_ELEMS + KV_V_ELEMS
NEG = -1e30

BF16 = ml_dtypes.bfloat16

_RUNNER = None


def _build_nc():
    import concourse.mybir as mybir
    import concourse.tile as tile
    from concourse import bacc
    from contextlib import ExitStack

    f32 = mybir.dt.float32
    bf16 = mybir.dt.bfloat16

    nc = bacc.Bacc("TRN2", target_bir_lowering=False, debug=False,
                   num_devices=NCORES)

    xt_p = nc.declare_dram_parameter("xt", [E, MT], bf16, isOutput=False)
    wqkvt_p = nc.declare_dram_parameter("wqkvt", [L * E, 3 * E], bf16, isOutput=False)
    bqkv_p = nc.declare_dram_parameter("bqkv", [L * 3 * E], f32, isOutput=False)
    maska_p = nc.declare_dram_parameter("maska", [128, 1024], f32, isOutput=False)
    maskb_p = nc.declare_dram_parameter("maskb", [128, 2048], f32, isOutput=False)
    out_p = nc.declare_dram_parameter("out", [MT, N], f32, isOutput=True)

    AOP = mybir.AluOpType
    AF = mybir.ActivationFunctionType

    with tile.TileContext(nc) as tc:
        with ExitStack() as stack:
            ep_ = lambda **kw: stack.enter_context(tc.tile_pool(**kw))
            dram = ep_(name="dram", bufs=2, space="DRAM")
            consts = ep_(name="consts", bufs=1)
            px = ep_(name="px", bufs=2)
            pq = ep_(name="pq", bufs=2)
            pktf = ep_(name="pktf", bufs=1)
            pvf = ep_(name="pvf", bufs=1)
            pscore = ep_(name="pscore", bufs=1)
            pw = ep_(name="pw", bufs=1)
            pacc = ep_(name="pacc", bufs=1)
            pwqk = ep_(name="pwqk", bufs=8)
            pwv = ep_(name="pwv", bufs=2)
            pstage = ep_(name="pstage", bufs=4)
            pbias = ep_(name="pbias", bufs=2)
            pstats = ep_(name="pstats", bufs=4)
            psmm = ep_(name="psmm", bufs=2, space="PSUM")
            pssc = ep_(name="pssc", bufs=2, space="PSUM")
            pssm = ep_(name="pssm", bufs=2, space="PSUM")

            maska = consts.tile([128, 1024], f32)
            nc.sync.dma_start(maska[:], maska_p[:, :])
            maskb = consts.tile([128, 2048], f32)
            nc.sync.dma_start(maskb[:], maskb_p[:, :])
            zeros = consts.tile([128, 1024], f32)
            nc.vector.memset(zeros[:], 0.0)
            acc_a = pacc.tile([128, 1024], f32, tag="acca")
            nc.vector.memset(acc_a[:], 0.0)
            acc_b = pacc.tile([128, 2048], f32, tag="accb")
            nc.vector.memset(acc_b[:], 0.0)

            xt = px.tile([128, EC, MT], bf16, tag="xt")
            nc.sync.dma_start(
                xt[:], xt_p.ap().rearrange("(c p) m -> p c m", p=128))

            for li in range(L):
                last = li == L - 1
                wrow = li * E  # weight row offset for this layer

                bq = pbias.tile([128, 24], f32, tag="bq")
                nc.sync.dma_start(
                    bq[:],
                    bqkv_p.ap()[li * 3 * E:(li + 1) * 3 * E]
                    .rearrange("(c p) -> p c", p=128))

                kv_s = dram.tile([KV_ELEMS], bf16, tag="kvs")
                kv_d = dram.tile([KV_ELEMS * NCORES], bf16, tag="kvd")

                # ---- K projection (features 1024:2048 -> f_tiles 8..15) ----
                for ft in range(8, 16):
                    wt = pwqk.tile([128, EC, 128], bf16, tag="wqk")
                    nc.sync.dma_start(
                        wt[:],
                        wqkvt_p.ap()[wrow:wrow + E, 128 * ft:128 * (ft + 1)]
                        .rearrange("(c p) f -> p c f", p=128))
                    ps = psmm.tile([128, MT], f32, tag="mm")
                    for ec in range(EC):
                        nc.tensor.matmul(ps[:], wt[:, ec, :], xt[:, ec, :],
                                         start=(ec == 0), stop=(ec == EC - 1))
                    kst = pstage.tile([128, MT], bf16, tag="kst")
                    nc.scalar.activation(kst[:], ps[:], AF.Identity,
                                         bias=bq[:, ft:ft + 1])
                    kt = ft - 8
                    nc.sync.dma_start(
                        kv_s[kt * 128 * MT:(kt + 1) * 128 * MT]
                        .rearrange("(p m) -> p m", p=128),
                        kst[:])

                # ---- V projection (natural layout [m, e]) ----
                if not last:
                    for s in range(2):
                        wvt_w = pwv.tile([128, EC, 512], bf16, tag="wv")
                        nc.sync.dma_start(
                            wvt_w[:],
                            wqkvt_p.ap()[wrow:wrow + E,
                                         2048 + 512 * s:2048 + 512 * (s + 1)]
                            .rearrange("(c p) f -> p c f", p=128))
                        for mt in range(2):
                            ps = psmm.tile([128, 512], f32, tag="mm")
                            for ec in range(EC):
                                nc.tensor.matmul(
                                    ps[:], xt[:, ec, 128 * mt:128 * (mt + 1)],
                                    wvt_w[:, ec, :],
                                    start=(ec == 0), stop=(ec == EC - 1))
                            vst = pstage.tile([128, 512], bf16, tag="vst")
                            nc.scalar.copy(vst[:], ps[:])
                            base = KV_K_ELEMS + mt * 128 * E
                            nc.sync.dma_start(
                                kv_s[base:base + 128 * E]
                                .rearrange("(p e) -> p e", p=128)
                                [:, 512 * s:512 * (s + 1)],
                                vst[:])

                # ---- all-gather K,V ----
                nc.gpsimd.collective_compute(
                    "AllGather", AOP.bypass,
                    replica_groups=[list(range(NCORES))],
                    ins=[kv_s[:].opt()],
                    outs=[kv_d[:].opt()],
                )

                # ---- Q projection (features 0:1024, pre-scaled weights) ----
                qt = pq.tile([128, EC, MT], bf16, tag="qt")
                for ft in range(8):
                    wt = pwqk.tile([128, EC, 128], bf16, tag="wqk")
                    nc.sync.dma_start(
                        wt[:],
                        wqkvt_p.ap()[wrow:wrow + E, 128 * ft:128 * (ft + 1)]
                        .rearrange("(c p) f -> p c f", p=128))
                    ps = psmm.tile([128, MT], f32, tag="mm")
                    for ec in range(EC):
                        nc.tensor.matmul(ps[:], wt[:, ec, :], xt[:, ec, :],
                                         start=(ec == 0), stop=(ec == EC - 1))
                    nc.scalar.activation(qt[:, ft, :], ps[:], AF.Identity,
                                         bias=bq[:, ft:ft + 1])

                # ---- unpack gathered K (one wide DMA per column half) ----
                # columns 0:1024 = every rank's first local row-tile (A halves,
                # global tiles 0..7); columns 1024:2048 = B halves in reverse
                # rank order (global tiles 8..15 <- ranks 7..0).
                ktfa = pktf.tile([128, EC, 1024], bf16, tag="ktfa")
                ktfb = pktf.tile([128, EC, 1024], bf16, tag="ktfb")
                for r in range(NCORES):
                    src = (kv_d[r * KV_ELEMS:r * KV_ELEMS + KV_K_ELEMS]
                           .rearrange("(c p m) -> p c m", p=128, m=MT))
                    nc.sync.dma_start(ktfa[:, :, 128 * r:128 * (r + 1)],
                                      src[:, :, 0:128])
                    nc.sync.dma_start(ktfb[:, :, 128 * (7 - r):128 * (8 - r)],
                                      src[:, :, 128:256])

                # ---- scores + softmax + accumulate, per m-tile ----
                w_a = pw.tile([128, 1024], bf16, tag="wa")
                w_b = pw.tile([128, 2048], bf16, tag="wb")
                sm_jobs = []
                for mt, width, mask_t, w_t, acc_t, stag in (
                    (0, 1024, maska, w_a, acc_a, "a"),
                    (1, 2048, maskb, w_b, acc_b, "b"),
                ):
                    scores = pscore.tile([128, width], f32, tag=f"sc{stag}")
                    for ns in range(width // 512):
                        ktf_h = ktfa if ns < 2 else ktfb
                        ps = pssc.tile([128, 512], f32, tag="sc")
                        for ec in range(EC):
                            nc.tensor.matmul(
                                ps[:], qt[:, ec, 128 * mt:128 * (mt + 1)],
                                ktf_h[:, ec, 512 * (ns % 2):512 * (ns % 2 + 1)],
                                start=(ec == 0), stop=(ec == EC - 1))
                        nc.vector.scalar_tensor_tensor(
                            out=scores[:, 512 * ns:512 * (ns + 1)],
                            in0=ps[:], scalar=1.0,
                            in1=mask_t[:, 512 * ns:512 * (ns + 1)],
                            op0=AOP.mult, op1=AOP.add)
                    expv = pscore.tile([128, width], bf16, tag=f"ex{stag}")
                    rowsum = pstats.tile([128, 1], f32, tag="rs")
                    nc.scalar.activation(expv[:], scores[:], AF.Exp,
                                         accum_out=rowsum[:])
                    recip = pstats.tile([128, 1], f32, tag="rc")
                    nc.vector.reciprocal(recip[:], rowsum[:])
                    nc.vector.tensor_scalar_mul(w_t[:], expv[:], recip[:])
                    nc.vector.scalar_tensor_tensor(
                        out=acc_t[:], in0=w_t[:], scalar=1.0, in1=acc_t[:],
                        op0=AOP.mult, op1=AOP.add)

                if last:
                    continue

                # ---- unpack gathered V into natural [n, e] tiles ----
                vfa = pvf.tile([128, 8, E], bf16, tag="vfa")
                vfb = pvf.tile([128, 8, E], bf16, tag="vfb")
                for t in range(16):
                    r = t if t < 8 else 15 - t
                    mt = 0 if t < 8 else 1
                    vf_h, sl = (vfa, t) if t < 8 else (vfb, t - 8)
                    base = r * KV_ELEMS + KV_K_ELEMS + mt * 128 * E
                    nc.sync.dma_start(
                        vf_h[:, sl, :],
                        kv_d[base:base + 128 * E]
                        .rearrange("(p e) -> p e", p=128))

                # ---- transpose W tiles via DMA transpose (off the PE) ----
                # slot layout: 2t = tile-A chunk t (zero for t>=8), 2t+1 =
                # tile-B chunk t.
                wtr = pw.tile([128, 32, 128], bf16, tag="wt")
                nc.gpsimd.memset(wtr[:, 16:32, :], 0.0)
                for t in range(8):
                    nc.sync.dma_start_transpose(
                        wtr[:, 2 * t, :], w_a[:, 128 * t:128 * (t + 1)])
                for t in range(16):
                    nc.sync.dma_start_transpose(
                        wtr[:, 2 * t + 1, :], w_b[:, 128 * t:128 * (t + 1)])

                # ---- W @ V -> next layer activation (out-proj folded) ----
                xt_next = px.tile([128, EC, MT], bf16, tag="xt")
                for ep2 in range(EC):
                    ps = pssm.tile([128, MT], f32, tag="sm")
                    for t in range(16):
                        vf_h, sl = (vfa, t) if t < 8 else (vfb, t - 8)
                        nc.tensor.matmul(
                            ps[:], vf_h[:, sl, 128 * ep2:128 * (ep2 + 1)],
                            wtr[:, 2 * t:2 * t + 2, :],
                            start=(t == 0), stop=(t == 15))
                    nc.scalar.copy(xt_next[:, ep2, :], ps[:])
                xt = xt_next

            # ---- finalize: mean over layers, write output ----
            out_a = pscore.tile([128, 1024], f32, tag="sca")
            nc.scalar.mul(out_a[:], acc_a[:], 1.0 / L)
            nc.sync.dma_start(out_p[0:128, 0:1024], out_a[:])
            nc.sync.dma_start(out_p[0:128, 1024:2048], zeros[:])
            out_b = pscore.tile([128, 2048], f32, tag="scb")
            nc.scalar.mul(out_b[:], acc_b[:], 1.0 / L)
            nc.sync.dma_start(out_p[128:256, :], out_b[:])

    nc.compile()
    return nc


def _prep_in_maps(all_mentions, Wqkv, bqkv, Wo, bo):
    all_mentions = np.asarray(all_mentions, np.float32)
    Wqkv = np.asarray(Wqkv, np.float32)
    bqkv = np.asarray(bqkv, np.float32)
    Wo = np.asarray(Wo, np.float32)
    bo = np.asarray(bo, np.float32)

    # Fold each layer's output projection into the next layer's QKV:
    # x_i = wv_{i-1} @ Wo_{i-1}^T + bo_{i-1}
    # qkv_i = x_i @ Wqkv_i^T + bqkv_i
    #       = wv_{i-1} @ (Wqkv_i @ Wo_{i-1})^T + (bqkv_i + Wqkv_i @ bo_{i-1})
    Wp = np.empty_like(Wqkv)
    bp = np.empty_like(bqkv)
    Wp[0] = Wqkv[0]
    bp[0] = bqkv[0]
    for i in range(1, L):
        Wp[i] = Wqkv[i] @ Wo[i - 1]
        bp[i] = bqkv[i] + Wqkv[i] @ bo[i - 1]
    Wp[:, :E, :] *= SCALE   # torch scales Q by head_dim**-0.5
    bp[:, :E] *= SCALE

    wqkvt = np.ascontiguousarray(
        Wp.transpose(0, 2, 1)).reshape(L * E, 3 * E).astype(BF16)
    bqkv_flat = np.ascontiguousarray(bp.reshape(-1), np.float32)

    p = np.arange(128)
    j1 = np.arange(1024)
    j2 = np.arange(2048)

    in_maps = []
    for c in range(NCORES):
        ta, tb = c, 15 - c
        rows = np.concatenate([np.arange(128 * ta, 128 * (ta + 1)),
                               np.arange(128 * tb, 128 * (tb + 1))])
        xt = np.ascontiguousarray(all_mentions[rows].T).astype(BF16)
        maska = np.where(j1[None, :] <= (128 * ta + p)[:, None],
                         np.float32(0.0), np.float32(NEG)).astype(np.float32)
        maskb = np.where(j2[None, :] <= (128 * tb + p)[:, None],
                         np.float32(0.0), np.float32(NEG)).astype(np.float32)
        in_maps.append({
            "xt": xt,
            "wqkvt": wqkvt,
            "bqkv": bqkv_flat,
            "maska": maska,
            "maskb": maskb,
        })
    return in_maps


class Runner:
    def __init__(self):
        self.nc = _build_nc()

    def run(self, in_maps, **kw):
        from concourse.bass_utils import run_bass_kernel_spmd
        return run_bass_kernel_spmd(self.nc, in_maps,
                                    core_ids=list(range(NCORES)), **kw)


def get_runner():
    global _RUNNER
    if _RUNNER is None:
        _RUNNER = Runner()
    return _RUNNER


def assemble_output(results):
    out = np.zeros((N, N), np.float32)
    for c in range(NCORES):
        o = np.asarray(results[c]["out"], np.float32)
        out[128 * c:128 * (c + 1), :1024] = o[0:128, :1024]
        out[128 * (15 - c):128 * (16 - c), :] = o[128:256, :]
    return out


def kernel(all_mentions, Wqkv, bqkv, Wo, bo):
    runner = get_runner()
    in_maps = _prep_in_maps(all_mentions, Wqkv, bqkv, Wo, bo)
    res = runner.run(in_maps)
    return assemble_output(res.results)


# revision 9
# speedup vs baseline: 1.1351x; 1.1351x over previous
"""Distributed Trainium2 kernel for the 4-layer single-head causal-attention
stack (returns mean attention weights over layers).

Sharding: sequence-parallel over the 2048 mentions. 16 row-tiles of 128;
core c owns tiles {c, 15-c} so causal-attention work is identical on every
core -> one uniform SPMD program. Per layer each core projects K,V for its
256 rows, all-gathers K,V across the 8 cores (one collective), projects Q
while the gather runs, then computes masked scores, softmax and W@V in
bf16 with f32 PSUM accumulation.

The per-layer output projection is folded into the next layer's QKV
weights on the host (W'_i = Wqkv_i @ Wo_{i-1}), so x_i never materializes
on device - the W@V output feeds the next layer's projections directly.
Layer 3 only computes Q,K (its attention output is never consumed).

DMA traffic is batched into a handful of wide strided transfers per layer
(the HWDGE dispatch queues serialize per-DMA overhead, so DMA count
matters as much as bytes), split across both HWDGE rings (sync+scalar).
W^T for the W@V matmul comes from one batched DMA transpose of an
interleaved [A0 B0 A1 B1 ...] buffer, keeping the transposes off the
TensorEngine and giving N=256 moving operands in W@V.
"""

import numpy as np
import ml_dtypes

N, E, L, NCORES = 2048, 1024, 4, 8
EC = E // 128          # 8 contraction chunks of 128
MT = 256               # mention rows per core
SCALE = 1.0 / np.sqrt(np.float32(E))
KV_K_ELEMS = E * MT            # k block: [1024, 256] (feature-major)
KV_V_ELEMS = MT * E            # v block: [256, 1024] (row-major natural)
KV_ELEMS = KV_K_ELEMS + KV_V_ELEMS
NEG = -1e30

BF16 = ml_dtypes.bfloat16

_RUNNER = None


def _build_nc():
    import concourse.mybir as mybir
    import concourse.tile as tile
    from concourse import bacc
    from contextlib import ExitStack

    f32 = mybir.dt.float32
    bf16 = mybir.dt.bfloat16

    nc = bacc.Bacc("TRN2", target_bir_lowering=False, debug=False,
                   num_devices=NCORES)

    xt_p = nc.declare_dram_parameter("xt", [E, MT], bf16, isOutput=False)
    wqkvt_p = nc.declare_dram_parameter("wqkvt", [L * E, 3 * E], bf16, isOutput=False)
    bqkv_p = nc.declare_dram_parameter("bqkv", [L * 3 * E], f32, isOutput=False)
    maska_p = nc.declare_dram_parameter("maska", [128, 1024], bf16, isOutput=False)
    maskb_p = nc.declare_dram_parameter("maskb", [128, 2048], bf16, isOutput=False)
    out_p = nc.declare_dram_parameter("out", [MT, N], f32, isOutput=True)

    AOP = mybir.AluOpType
    AF = mybir.ActivationFunctionType

    with tile.TileContext(nc) as tc:
        with ExitStack() as stack:
            ep_ = lambda **kw: stack.enter_context(tc.tile_pool(**kw))
            dram = ep_(name="dram", bufs=2, space="DRAM")
            consts = ep_(name="consts", bufs=1)
            px = ep_(name="px", bufs=2)
            pq = ep_(name="pq", bufs=2)
            pktf = ep_(name="pktf", bufs=1)
            pvf = ep_(name="pvf", bufs=1)
            pscore = ep_(name="pscore", bufs=1)
            pw = ep_(name="pw", bufs=1)
            pacc = ep_(name="pacc", bufs=1)
            pwqk = ep_(name="pwqk", bufs=2)
            pwv = ep_(name="pwv", bufs=2)
            pstage = ep_(name="pstage", bufs=2)
            pbias = ep_(name="pbias", bufs=2)
            pstats = ep_(name="pstats", bufs=4)
            psmm = ep_(name="psmm", bufs=2, space="PSUM")
            pssc = ep_(name="pssc", bufs=2, space="PSUM")
            pssm = ep_(name="pssm", bufs=2, space="PSUM")

            maska = consts.tile([128, 1024], bf16)
            nc.sync.dma_start(maska[:], maska_p[:, :])
            maskb = consts.tile([128, 2048], bf16)
            nc.sync.dma_start(maskb[:], maskb_p[:, :])
            zeros = consts.tile([128, 1024], f32)
            nc.vector.memset(zeros[:], 0.0)
            acc_a = pacc.tile([128, 1024], f32, tag="acca")
            nc.vector.memset(acc_a[:], 0.0)
            acc_b = pacc.tile([128, 2048], f32, tag="accb")
            nc.vector.memset(acc_b[:], 0.0)

            # tiny warm-up collective: absorbs the ncfw cold-start latency
            # while the prologue DMAs run
            warm_s = dram.tile([256], bf16, tag="warms")
            warm_d = dram.tile([256 * NCORES], bf16, tag="warmd")
            warm_sb = consts.tile([128, 2], bf16)
            nc.vector.memset(warm_sb[:], 0.0)
            nc.sync.dma_start(
                warm_s[0:256].rearrange("(p m) -> p m", p=128), warm_sb[:])
            nc.gpsimd.collective_compute(
                "AllGather", AOP.bypass,
                replica_groups=[list(range(NCORES))],
                ins=[warm_s[:].opt()], outs=[warm_d[:].opt()])

            xt = px.tile([128, EC, MT], bf16, tag="xt")
            nc.sync.dma_start(
                xt[:], xt_p.ap().rearrange("(c p) m -> p c m", p=128))

            for li in range(L):
                last = li == L - 1
                wrow = li * E  # weight row offset for this layer

                bq = pbias.tile([128, 24], f32, tag="bq")
                nc.sync.dma_start(
                    bq[:],
                    bqkv_p.ap()[li * 3 * E:(li + 1) * 3 * E]
                    .rearrange("(c p) -> p c", p=128))

                kv_s = dram.tile([KV_ELEMS], bf16, tag="kvs")
                kv_d = dram.tile([KV_ELEMS * NCORES], bf16, tag="kvd")

                # ---- K projection (features 1024:2048 -> f_tiles 8..15) ----
                kstage = pstage.tile([128, 8, MT], bf16, tag="kst")
                for kw in range(2):
                    wt = pwqk.tile([128, EC, 512], bf16, tag="wqk")
                    nc.sync.dma_start(
                        wt[:],
                        wqkvt_p.ap()[wrow:wrow + E,
                                     1024 + 512 * kw:1024 + 512 * (kw + 1)]
                        .rearrange("(c p) f -> p c f", p=128))
                    for fl in range(4):
                        ft = 8 + 4 * kw + fl
                        ps = psmm.tile([128, MT], f32, tag="mm")
                        for ec in range(EC):
                            nc.tensor.matmul(
                                ps[:], wt[:, ec, 128 * fl:128 * (fl + 1)],
                                xt[:, ec, :],
                                start=(ec == 0), stop=(ec == EC - 1))
                        nc.scalar.activation(kstage[:, ft - 8, :], ps[:],
                                             AF.Identity, bias=bq[:, ft:ft + 1])
                nc.sync.dma_start(
                    kv_s[0:KV_K_ELEMS].rearrange("(c p m) -> p c m", p=128, m=MT),
                    kstage[:])

                # ---- V projection (natural layout [m, e]) ----
                if not last:
                    vstage = pstage.tile([128, 2, E], bf16, tag="vst")
                    for s in range(2):
                        wvt_w = pwv.tile([128, EC, 512], bf16, tag="wv")
                        nc.sync.dma_start(
                            wvt_w[:],
                            wqkvt_p.ap()[wrow:wrow + E,
                                         2048 + 512 * s:2048 + 512 * (s + 1)]
                            .rearrange("(c p) f -> p c f", p=128))
                        for mt in range(2):
                            ps = psmm.tile([128, 512], f32, tag="mm")
                            for ec in range(EC):
                                nc.tensor.matmul(
                                    ps[:], xt[:, ec, 128 * mt:128 * (mt + 1)],
                                    wvt_w[:, ec, :],
                                    start=(ec == 0), stop=(ec == EC - 1))
                            nc.scalar.copy(vstage[:, mt, 512 * s:512 * (s + 1)],
                                           ps[:])
                    nc.sync.dma_start(
                        kv_s[KV_K_ELEMS:KV_ELEMS]
                        .rearrange("(t p e) -> p t e", t=2, p=128),
                        vstage[:])

                # ---- all-gather K,V ----
                nc.gpsimd.collective_compute(
                    "AllGather", AOP.bypass,
                    replica_groups=[list(range(NCORES))],
                    ins=[kv_s[:].opt()],
                    outs=[kv_d[:].opt()],
                )

                # ---- Q projection (features 0:1024, pre-scaled weights) ----
                qt = pq.tile([128, EC, MT], bf16, tag="qt")
                for kw in range(2):
                    wt = pwqk.tile([128, EC, 512], bf16, tag="wqk")
                    nc.sync.dma_start(
                        wt[:],
                        wqkvt_p.ap()[wrow:wrow + E, 512 * kw:512 * (kw + 1)]
                        .rearrange("(c p) f -> p c f", p=128))
                    for fl in range(4):
                        ft = 4 * kw + fl
                        ps = psmm.tile([128, MT], f32, tag="mm")
                        for ec in range(EC):
                            nc.tensor.matmul(
                                ps[:], wt[:, ec, 128 * fl:128 * (fl + 1)],
                                xt[:, ec, :],
                                start=(ec == 0), stop=(ec == EC - 1))
                        nc.scalar.activation(qt[:, ft, :], ps[:], AF.Identity,
                                             bias=bq[:, ft:ft + 1])

                # ---- unpack gathered K (2 wide DMAs on the scalar ring) ----
                # columns 0:1024 = every rank's tile-A half (global tiles
                # 0..7); columns 1024:2048 = tile-B halves in reverse rank
                # order (global tile 15-r from rank r).
                kv2 = kv_d[:].rearrange("(r x) -> r x", r=NCORES)
                ktfa = pktf.tile([128, EC, 1024], bf16, tag="ktfa")
                ktfb = pktf.tile([128, EC, 1024], bf16, tag="ktfb")
                ksrc = kv2[:, 0:KV_K_ELEMS].rearrange(
                    "r (c p m) -> p c r m", p=128, m=MT)
                for r in range(NCORES):
                    nc.scalar.dma_start(ktfa[:, :, 128 * r:128 * (r + 1)],
                                        ksrc[:, :, r, 0:128])
                    nc.scalar.dma_start(ktfb[:, :, 128 * (7 - r):128 * (8 - r)],
                                        ksrc[:, :, r, 128:256])

                # ---- unpack gathered V (2 wide DMAs) ----
                if not last:
                    vfa = pvf.tile([128, 8, E], bf16, tag="vfa")
                    vfb = pvf.tile([128, 8, E], bf16, tag="vfb")
                    nc.scalar.dma_start(
                        vfa[:],
                        kv2[:, KV_K_ELEMS:KV_K_ELEMS + 128 * E]
                        .rearrange("r (p e) -> p r e", p=128))
                    nc.scalar.dma_start(
                        vfb[:].rearrange("p r e -> p r e")[:, ::-1, :],
                        kv2[:, KV_K_ELEMS + 128 * E:KV_ELEMS]
                        .rearrange("r (p e) -> p r e", p=128))

                # ---- scores + softmax + accumulate, per m-tile ----
                # w_ab interleaves [A0 B0 A1 B1 ... A15 B15] (A_t zero for
                # t>=8) so one batched DMA transpose yields the paired
                # [128, 32, 128] wT layout for N=256 W@V matmuls.
                if not last:
                    w_ab = pw.tile([128, 4096], bf16, tag="wab")
                    nc.gpsimd.memset(w_ab[:, 2048:4096], 0.0)
                for mt, width, mask_t, acc_t, stag in (
                    (0, 1024, maska, acc_a, "a"),
                    (1, 2048, maskb, acc_b, "b"),
                ):
                    scores = pscore.tile([128, width], f32, tag=f"sc{stag}")
                    for ns in range(width // 512):
                        ktf_h = ktfa if ns < 2 else ktfb
                        ps = pssc.tile([128, 512], f32, tag="sc")
                        for ec in range(EC):
                            nc.tensor.matmul(
                                ps[:], qt[:, ec, 128 * mt:128 * (mt + 1)],
                                ktf_h[:, ec, 512 * (ns % 2):512 * (ns % 2 + 1)],
                                start=(ec == 0), stop=(ec == EC - 1))
                        nc.vector.scalar_tensor_tensor(
                            out=scores[:, 512 * ns:512 * (ns + 1)],
                            in0=ps[:], scalar=1.0,
                            in1=mask_t[:, 512 * ns:512 * (ns + 1)],
                            op0=AOP.mult, op1=AOP.add)
                    expv = pscore.tile([128, width], bf16, tag=f"ex{stag}")
                    rowsum = pstats.tile([128, 1], f32, tag="rs")
                    nc.scalar.activation(expv[:], scores[:], AF.Exp,
                                         accum_out=rowsum[:])
                    recip = pstats.tile([128, 1], f32, tag="rc")
                    nc.vector.reciprocal(recip[:], rowsum[:])
                    # acc += expv * recip (fused; w never needed in f32)
                    nc.vector.scalar_tensor_tensor(
                        out=acc_t[:], in0=expv[:], scalar=recip[:],
                        in1=acc_t[:], op0=AOP.mult, op1=AOP.add)
                    if not last:
                        # normalized w into the interleaved buffer
                        nch = width // 128
                        w_view = (w_ab[:]
                                  .rearrange("p (t m) -> p t m", m=128)
                                  [:, mt:2 * nch:2, :])
                        nc.vector.tensor_scalar_mul(
                            w_view,
                            expv[:].rearrange("p (t m) -> p t m", m=128),
                            recip[:])

                if last:
                    continue

                # ---- one batched W^T transpose (off the PE) ----
                wtr = pw.tile([128, 32, 128], bf16, tag="wt")
                nc.sync.dma_start_transpose(wtr[:], w_ab[:])

                # ---- W @ V -> next layer activation (out-proj folded) ----
                xt_next = px.tile([128, EC, MT], bf16, tag="xt")
                for ep2 in range(EC):
                    ps = pssm.tile([128, MT], f32, tag="sm")
                    for t in range(16):
                        vf_h, sl = (vfa, t) if t < 8 else (vfb, t - 8)
                        nc.tensor.matmul(
                            ps[:], vf_h[:, sl, 128 * ep2:128 * (ep2 + 1)],
                            wtr[:, 2 * t:2 * t + 2, :],
                            start=(t == 0), stop=(t == 15))
                    nc.scalar.copy(xt_next[:, ep2, :], ps[:])
                xt = xt_next

            # ---- finalize: mean over layers, write output ----
            out_a = pscore.tile([128, 1024], f32, tag="sca")
            nc.scalar.mul(out_a[:], acc_a[:], 1.0 / L)
            nc.sync.dma_start(out_p[0:128, 0:1024], out_a[:])
            nc.sync.dma_start(out_p[0:128, 1024:2048], zeros[:])
            out_b = pscore.tile([128, 2048], f32, tag="scb")
            nc.scalar.mul(out_b[:], acc_b[:], 1.0 / L)
            nc.sync.dma_start(out_p[128:256, :], out_b[:])

    nc.compile()
    return nc


def _prep_in_maps(all_mentions, Wqkv, bqkv, Wo, bo):
    all_mentions = np.asarray(all_mentions, np.float32)
    Wqkv = np.asarray(Wqkv, np.float32)
    bqkv = np.asarray(bqkv, np.float32)
    Wo = np.asarray(Wo, np.float32)
    bo = np.asarray(bo, np.float32)

    # Fold each layer's output projection into the next layer's QKV:
    # qkv_i = wv_{i-1} @ (Wqkv_i @ Wo_{i-1})^T + (bqkv_i + Wqkv_i @ bo_{i-1})
    Wp = np.empty_like(Wqkv)
    bp = np.empty_like(bqkv)
    Wp[0] = Wqkv[0]
    bp[0] = bqkv[0]
    for i in range(1, L):
        Wp[i] = Wqkv[i] @ Wo[i - 1]
        bp[i] = bqkv[i] + Wqkv[i] @ bo[i - 1]
    Wp[:, :E, :] *= SCALE   # torch scales Q by head_dim**-0.5
    bp[:, :E] *= SCALE

    wqkvt = np.ascontiguousarray(
        Wp.transpose(0, 2, 1)).reshape(L * E, 3 * E).astype(BF16)
    bqkv_flat = np.ascontiguousarray(bp.reshape(-1), np.float32)

    p = np.arange(128)
    j1 = np.arange(1024)
    j2 = np.arange(2048)

    in_maps = []
    for c in range(NCORES):
        ta, tb = c, 15 - c
        rows = np.concatenate([np.arange(128 * ta, 128 * (ta + 1)),
                               np.arange(128 * tb, 128 * (tb + 1))])
        xt = np.ascontiguousarray(all_mentions[rows].T).astype(BF16)
        maska = np.where(j1[None, :] <= (128 * ta + p)[:, None],
                         np.float32(0.0), np.float32(NEG)).astype(BF16)
        maskb = np.where(j2[None, :] <= (128 * tb + p)[:, None],
                         np.float32(0.0), np.float32(NEG)).astype(BF16)
        in_maps.append({
            "xt": xt,
            "wqkvt": wqkvt,
            "bqkv": bqkv_flat,
            "maska": maska,
            "maskb": maskb,
        })
    return in_maps


class Runner:
    def __init__(self):
        self.nc = _build_nc()

    def run(self, in_maps, **kw):
        from concourse.bass_utils import run_bass_kernel_spmd
        return run_bass_kernel_spmd(self.nc, in_maps,
                                    core_ids=list(range(NCORES)), **kw)


def get_runner():
    global _RUNNER
    if _RUNNER is None:
        _RUNNER = Runner()
    return _RUNNER


def assemble_output(results):
    out = np.zeros((N, N), np.float32)
    for c in range(NCORES):
        o = np.asarray(results[c]["out"], np.float32)
        out[128 * c:128 * (c + 1), :1024] = o[0:128, :1024]
        out[128 * (15 - c):128 * (16 - c), :] = o[128:256, :]
    return out


def kernel(all_mentions, Wqkv, bqkv, Wo, bo):
    runner = get_runner()
    in_maps = _prep_in_maps(all_mentions, Wqkv, bqkv, Wo, bo)
    res = runner.run(in_maps)
    return assemble_output(res.results)
